# revision 4
# baseline (speedup 1.0000x reference)
"""DeepKMeans (vq_codebook) Trainium2 Bass kernel.

Computes, for x [8192, 2048], W_enc [2048, 256], W_dec [256, 2048],
cluster_reps [256, 256]:
    emb   = x @ W_enc + b_enc                       [B, K]
    recon = emb @ W_dec + b_dec                     [B, D]
    dist  = |emb|^2 - 2 emb @ C^T + |c_k|^2         [B, K]
    weighted = dist.T * softmin_k(alpha * dist.T)   [K, B]
returns (weighted, recon).

Sharding: data-parallel over the batch dim across 8 NeuronCores
(1024 rows each); weights replicated. x is transposed on the host so the
contraction dim lands on SBUF partitions; the weighted output is computed
in natural [B, K] layout on-device and transposed on the host.

Precision strategy: the distance path (emb, emb @ C^T, cn) runs in fp32
matmuls (softmin with alpha=1000 amplifies distance errors ~1000x); the
reconstruction matmul and the |emb|^2 row-sum run in float32r (13-mantissa
bit, 4x faster) since their errors are either benign (recon tolerance) or
cancel in the softmin (per-row shift invariance).
"""

import os
import sys

sys.path.insert(0, "/opt/trn_rl_repo")

import numpy as np

B, D, K = 8192, 2048, 256
NCORES = 8
BS = B // NCORES          # 1024 batch rows per core
KT = D // 128             # 16 contraction tiles for emb
MT = BS // 128            # 8 batch tiles per core
ALPHA = 1000.0

_prog_cache = {}


def _build_program(reps: int):
    import concourse.mybir as mybir
    import concourse.tile as tile
    from concourse import bacc

    f32 = mybir.dt.float32
    f32r = mybir.dt.float32r

    nc = bacc.Bacc()
    XT = nc.declare_dram_parameter("XT", [D, BS], f32, isOutput=False)
    WE = nc.declare_dram_parameter("WE", [D, K], f32, isOutput=False)
    BE = nc.declare_dram_parameter("BE", [K], f32, isOutput=False)
    WD = nc.declare_dram_parameter("WD", [K, D], f32r, isOutput=False)
    BD = nc.declare_dram_parameter("BD", [1, D], f32, isOutput=False)
    M2CT = nc.declare_dram_parameter("M2CT", [K, K], f32, isOutput=False)
    CN = nc.declare_dram_parameter("CN", [1, K], f32, isOutput=False)
    RECON = nc.declare_dram_parameter("RECON", [BS, D], f32, isOutput=True)
    WGT = nc.declare_dram_parameter("WGT", [BS, K], f32, isOutput=True)

    with tile.TileContext(nc) as tc:
        with (
            tc.tile_pool(name="big", bufs=1) as big,
            tc.tile_pool(name="emb", bufs=1) as embp,
            tc.tile_pool(name="cst", bufs=1) as cst,
            tc.tile_pool(name="out", bufs=6) as outp,
            tc.tile_pool(name="sm", bufs=8) as sm,
            tc.tile_pool(name="mm", bufs=4, space="PSUM") as mmp,
            tc.tile_pool(name="dd", bufs=2, space="PSUM") as ddp,
        ):
            # ---- load inputs ----
            xt_sb = big.tile([128, KT, BS], f32)
            xt_src = XT.ap().rearrange("(t p) n -> t p n", p=128)
            for t in range(KT):
                nc.sync.dma_start(out=xt_sb[:, t, :], in_=xt_src[t])
            we_sb = cst.tile([128, KT, K], f32)
            we_src = WE.ap().rearrange("(t p) n -> t p n", p=128)
            for t in range(KT):
                nc.sync.dma_start(out=we_sb[:, t, :], in_=we_src[t])
            wd_sb = cst.tile([128, 2, D], f32r)
            wd_src = WD.ap().rearrange("(t p) n -> t p n", p=128)
            for t in range(2):
                nc.sync.dma_start(out=wd_sb[:, t, :], in_=wd_src[t])
            m2ct_sb = cst.tile([128, 2, K], f32)
            nc.sync.dma_start(
                out=m2ct_sb, in_=M2CT.ap().rearrange("(t p) n -> p t n", p=128)
            )
            be_sb = cst.tile([128, 2], f32)
            nc.sync.dma_start(out=be_sb, in_=BE.ap().rearrange("(t p) -> p t", p=128))
            bd_sb = cst.tile([128, D], f32)
            nc.sync.dma_start(out=bd_sb, in_=BD.ap().to_broadcast((128, D)))
            cn_sb = cst.tile([128, K], f32)
            nc.sync.dma_start(out=cn_sb, in_=CN.ap().to_broadcast((128, K)))
            ones_f = cst.tile([128, K], f32)
            nc.vector.memset(ones_f, 1.0)
            ones_r = cst.tile([128, K], f32r)
            nc.vector.tensor_copy(ones_r, ones_f)

            for _ in range(reps):
                # ---- phase B: embT[k2, b] = (x @ W_enc + b_enc)^T, fp32 ----
                embT = embp.tile([128, 2, BS], f32, tag="embT")
                for m in range(2):
                    psums = [mmp.tile([128, 512], f32, tag="mm", name=f"psb{m}_{i}") for i in range(2)]
                    for k in range(KT):
                        for n in range(2):
                            nc.tensor.matmul(
                                psums[n],
                                we_sb[:, k, m * 128:(m + 1) * 128],
                                xt_sb[:, k, n * 512:(n + 1) * 512],
                                start=(k == 0),
                                stop=(k == KT - 1),
                            )
                    for n in range(2):
                        nc.vector.tensor_scalar_add(
                            embT[:, m, n * 512:(n + 1) * 512],
                            psums[n],
                            be_sb[:, m:m + 1],
                        )

                # rounded copies for the f32r passes
                emb_r = embp.tile([128, 2, BS], f32r, tag="emb_r")
                sq_r = embp.tile([128, 2, BS], f32r, tag="sq_r")
                for m in range(2):
                    nc.vector.tensor_copy(emb_r[:, m, :], embT[:, m, :])
                    nc.vector.tensor_mul(sq_r[:, m, :], embT[:, m, :], embT[:, m, :])

                # ---- phase C: recon = emb @ W_dec + b_dec, f32r ----
                for mb in range(MT):
                    psums = [mmp.tile([128, 512], f32, tag="mm", name=f"psc{mb}_{i}") for i in range(4)]
                    for k2 in range(2):
                        for nd in range(4):
                            nc.tensor.matmul(
                                psums[nd],
                                emb_r[:, k2, mb * 128:(mb + 1) * 128],
                                wd_sb[:, k2, nd * 512:(nd + 1) * 512],
                                start=(k2 == 0),
                                stop=(k2 == 1),
                            )
                    for nd in range(4):
                        ro = outp.tile([128, 512], f32, tag="ro")
                        nc.vector.tensor_add(
                            ro, psums[nd], bd_sb[:, nd * 512:(nd + 1) * 512]
                        )
                        nc.sync.dma_start(
                            out=RECON.ap()[
                                mb * 128:(mb + 1) * 128, nd * 512:(nd + 1) * 512
                            ],
                            in_=ro,
                        )

                # ---- phase D/F: dist + softmin, natural [b, k] layout ----
                for mb in range(MT):
                    dps = ddp.tile([128, K], f32, tag="dd")
                    bsl = slice(mb * 128, (mb + 1) * 128)
                    # -2 emb @ C^T  (fp32)
                    for j in range(2):
                        nc.tensor.matmul(
                            dps,
                            embT[:, j, bsl],
                            m2ct_sb[:, j, :],
                            start=(j == 0),
                            stop=False,
                        )
                    # + |emb_b|^2 broadcast over k (f32r; error cancels in softmin)
                    for j in range(2):
                        nc.tensor.matmul(
                            dps,
                            sq_r[:, j, bsl],
                            ones_r,
                            start=False,
                            stop=(j == 1),
                        )
                    # dist = dps + |c_k|^2
                    dist = outp.tile([128, K], f32, tag="dist")
                    nc.vector.tensor_add(dist, dps, cn_sb)
                    m_col = sm.tile([128, 1], f32, tag="mcol")
                    nc.vector.tensor_reduce(
                        out=m_col, in_=dist, axis=mybir.AxisListType.X,
                        op=mybir.AluOpType.min,
                    )
                    bias_col = sm.tile([128, 1], f32, tag="bcol")
                    nc.vector.tensor_scalar_mul(bias_col, m_col, ALPHA)
                    e_sb = outp.tile([128, K], f32, tag="esb")
                    z_col = sm.tile([128, 1], f32, tag="zcol")
                    nc.scalar.activation(
                        out=e_sb,
                        in_=dist,
                        func=mybir.ActivationFunctionType.Exp,
                        bias=bias_col,
                        scale=-ALPHA,
                        accum_out=z_col,
                    )
                    rz_col = sm.tile([128, 1], f32, tag="rzcol")
                    nc.vector.reciprocal(rz_col, z_col)
                    w1 = outp.tile([128, K], f32, tag="w1")
                    nc.vector.tensor_mul(w1, dist, e_sb)
                    wout = outp.tile([128, K], f32, tag="wout")
                    nc.vector.tensor_scalar_mul(wout, w1, rz_col)
                    nc.sync.dma_start(out=WGT.ap()[bsl, :], in_=wout)

    nc.finalize()
    return nc


def _get_program(reps: int):
    if reps not in _prog_cache:
        _prog_cache[reps] = _build_program(reps)
    return _prog_cache[reps]


def kernel(x, W_enc, b_enc, W_dec, b_dec, cluster_reps):
    from concourse.bass_utils import run_bass_kernel_spmd

    x = np.ascontiguousarray(np.asarray(x, dtype=np.float32))
    W_enc = np.ascontiguousarray(np.asarray(W_enc, dtype=np.float32))
    b_enc = np.ascontiguousarray(np.asarray(b_enc, dtype=np.float32))
    W_dec = np.ascontiguousarray(np.asarray(W_dec, dtype=np.float32))
    b_dec = np.ascontiguousarray(np.asarray(b_dec, dtype=np.float32))
    cluster_reps = np.ascontiguousarray(np.asarray(cluster_reps, dtype=np.float32))

    m2ct = np.ascontiguousarray(-2.0 * cluster_reps.T)
    cn = (cluster_reps * cluster_reps).sum(axis=1, dtype=np.float32).reshape(1, K)
    cn = np.ascontiguousarray(cn)

    reps = int(os.environ.get("BASS_KERNEL_REPS", "1"))
    nc = _get_program(reps)

    in_maps = []
    for c in range(NCORES):
        xs = x[c * BS:(c + 1) * BS, :]
        in_maps.append({
            "XT": np.ascontiguousarray(xs.T),
            "WE": W_enc,
            "BE": b_enc,
            "WD": W_dec,
            "BD": b_dec.reshape(1, D),
            "M2CT": m2ct,
            "CN": cn,
        })

    res = run_bass_kernel_spmd(nc, in_maps, list(range(NCORES)))

    recon = np.concatenate([r["RECON"] for r in res.results], axis=0)
    wnat = np.concatenate([r["WGT"] for r in res.results], axis=0)
    weighted = np.ascontiguousarray(wnat.T)
    return weighted, recon
